# revision 53
# baseline (speedup 1.0000x reference)
"""Trainium2 Bass kernel for nn_Attention_41686952575399 (sparse attention).

Sharding: data-parallel over batch (2 groups of 4 cores) x tensor-parallel over
heads (4 heads per core). Per-head device-side AllGather chunks (bf16) overlap
with the next head's compute; each core then computes a 256-wide dout slice of
the output projection for all tokens of its batch element.

vs the fp32r baseline:
  - bf16 datapath for all matmul operands (psum accumulation stays fp32);
    halves HBM loads and collective bytes.
  - rms_w / conv positional embedding / conv bias folded on the host.
  - compression conv as 8 accumulating matmuls with (dh, token-parity) packed
    128-partition contraction instead of 16 64-contraction matmuls.
  - window P@V via 8 matmuls into two 256-col-span psum accumulators (even /
    odd key tiles, odd spans shifted 128 cols) instead of 15 128-col matmuls.
  - paired PE transposes produce V in natural layout for two heads at once.
  - reciprocal_approx_fast for softmax denominators and RMS norm.
  - per-head AllGather issued right after each head's mix -> only the last
    ~quarter of the collective is exposed.
"""
import os
import sys

sys.path.insert(0, "/opt/trn_rl_repo")

DEBUG = os.environ.get("BASSK_DEBUG") == "1"

import numpy as np
import ml_dtypes

from concourse import bacc, bass, mybir, tile
from concourse.bass_utils import run_bass_kernel_spmd

B, N, DIM = 2, 1024, 1024
H, DH = 16, 64
WIN, CB = 64, 16
NB = N // CB               # 64 compressed blocks
HPC = 4                    # heads per core
NCORES = 8
GROUPS = [[0, 1, 2, 3], [4, 5, 6, 7]]
F32 = mybir.dt.float32
BF16 = mybir.dt.bfloat16
NEG = -1e30
EPS = float(np.finfo(np.float32).eps)
SCALE = float(DH ** -0.5)
NF = 3 * HPC * DH + 3      # 771 projection output features (q,k,v slices + Ws)
NPB = ml_dtypes.bfloat16

AL = mybir.AluOpType
AF = mybir.ActivationFunctionType


def build_program() -> bass.Bass:
    nc = bacc.Bacc("TRN2", target_bir_lowering=False, debug=False,
                   num_devices=NCORES)

    inpT_d = nc.dram_tensor("inpT", [DIM, N], BF16, kind="ExternalInput")
    wall_d = nc.dram_tensor("w_all", [DIM, NF], BF16, kind="ExternalInput")
    cwk_d = nc.dram_tensor("cw_k", [128, 2, CB, 128], BF16, kind="ExternalInput")
    cwv_d = nc.dram_tensor("cw_v", [128, 2, CB, 128], BF16, kind="ExternalInput")
    kcb_d = nc.dram_tensor("kcb", [128, 2], F32, kind="ExternalInput")
    vcb_d = nc.dram_tensor("vcb", [128, 2], F32, kind="ExternalInput")
    bs_d = nc.dram_tensor("bs_t", [3, 1], F32, kind="ExternalInput")
    wout_d = nc.dram_tensor("woutS", [128, HPC, 2, 256], BF16, kind="ExternalInput")
    ones_d = nc.dram_tensor("ones_c", [128, 8], BF16, kind="ExternalInput")
    ident_d = nc.dram_tensor("ident_c", [128, 128], BF16, kind="ExternalInput")
    outT_d = nc.dram_tensor("outT", [256, N], F32, kind="ExternalOutput")
    dbg = {}
    if DEBUG:
        dbg["s"] = nc.dram_tensor("dbg_s", [1, N], F32, kind="ExternalOutput")
        dbg["w3"] = nc.dram_tensor("dbg_w3", [3, N], F32, kind="ExternalOutput")
        dbg["qk"] = nc.dram_tensor("dbg_qk", [128, 2, 2, N], BF16,
                                   kind="ExternalOutput")
        dbg["v2"] = nc.dram_tensor("dbg_v2", [128, 2, N], BF16,
                                   kind="ExternalOutput")
        dbg["ck"] = nc.dram_tensor("dbg_ck", [128, DH], BF16, kind="ExternalOutput")
        dbg["cva"] = nc.dram_tensor("dbg_cva", [DH, DH + 1], BF16,
                                    kind="ExternalOutput")
        dbg["pc"] = nc.dram_tensor("dbg_pc", [NB, 2, N], BF16, kind="ExternalOutput")
        dbg["pw"] = nc.dram_tensor("dbg_pw", [128, 2, 8, 256], BF16,
                                   kind="ExternalOutput")
        dbg["vnat"] = nc.dram_tensor("dbg_vnat", [128, 2, 8, 130], BF16,
                                     kind="ExternalOutput")
        dbg["den"] = nc.dram_tensor("dbg_den", [2, N], F32, kind="ExternalOutput")
        dbg["denr"] = nc.dram_tensor("dbg_denr", [2, N], F32, kind="ExternalOutput")
        dbg["comb"] = nc.dram_tensor("dbg_comb", [DH, HPC, N], BF16,
                                     kind="ExternalOutput")
        dbg["cmb"] = nc.dram_tensor("dbg_cmb", [128, HPC, 2, N], BF16,
                                    kind="ExternalOutput")

    with tile.TileContext(nc) as tc:
        _body(nc, tc, inpT_d, wall_d, cwk_d, cwv_d, kcb_d, vcb_d, bs_d,
              wout_d, ones_d, ident_d, outT_d, dbg)
    nc.compile()
    return nc


def _body(nc, tc, inpT_d, wall_d, cwk_d, cwv_d, kcb_d, vcb_d, bs_d,
          wout_d, ones_d, ident_d, outT_d, dbg):
    mm = nc.tensor.matmul

    # ----- long-lived constants -----------------------------------------
    const_cm = tc.tile_pool(name="const", bufs=1)
    const = const_cm.__enter__()
    ones_b = const.tile([128, 8], BF16, name="ones_b")
    ident = const.tile([128, 128], BF16, name="ident")
    cmask = const.tile([NB, N], F32, name="cmask")
    wmask = const.tile([128, 192], F32, name="wmask")
    bs_sb = const.tile([3, 1], F32, name="bs_sb")
    kcb_sb = const.tile([128, 2], F32, name="kcb_sb")
    vcb_sb = const.tile([128, 2], F32, name="vcb_sb")
    eps_sb = const.tile([1, 1], F32, name="eps_sb")
    s_srt = const.tile([1, N], F32, name="s_srt")
    s_row = const.tile([1, N], F32, name="s_row")
    s_bcast = const.tile([128, N], F32, name="s_bcast")
    w3r = const.tile([3, N], F32, name="w3r")
    w3_sb = const.tile([3, N], F32, name="w3_sb")
    w1_row = const.tile([1, N], F32, name="w1_row")
    wout_sb = const.tile([128, HPC, 2, 256], BF16, name="wout_sb")
    cwk_sb = const.tile([128, 2, CB, 128], BF16, name="cwk_sb")
    cwv_sb = const.tile([128, 2, CB, 128], BF16, name="cwv_sb")

    # ----- stage 1+2: RMS stats + fused qkv/Ws projection ---------------
    # qk2: q/k with even head of the pair on partitions 0-63, odd on 64-127.
    qk2, qk2_free = tc.tile([128, 2, 2, N], BF16, name="qk2")
    v2, v2_free = tc.tile([128, 2, N], BF16, name="v2")
    q_od, q_od_free = tc.tile([DH, 2, N], BF16, name="q_od")
    k_od, k_od_free = tc.tile([DH, 2, N], BF16, name="k_od")
    x_sb, x_free = tc.tile([128, 8, N], BF16, name="x_sb")
    w_sb, w_free = tc.tile([128, 8, NF], BF16, name="w_sb")

    # x/w loads first (they gate all compute), split across two queues
    for k in range(8):
        nc.gpsimd.dma_start(out=x_sb[:, k, :], in_=inpT_d.ap()[128 * k:128 * (k + 1), :])
        nc.sync.dma_start(out=w_sb[:, k, :], in_=wall_d.ap()[128 * k:128 * (k + 1), :])
    nc.gpsimd.dma_start(out=ones_b[:], in_=ones_d.ap())
    nc.gpsimd.dma_start(out=ident[:], in_=ident_d.ap())
    nc.gpsimd.memset(eps_sb[:], EPS)
    # compressed-block causal mask: block c visible to token t iff t >= 16c+15
    nc.gpsimd.memset(cmask[:], 0.0)
    nc.gpsimd.affine_select(out=cmask[:], in_=cmask[:], compare_op=AL.is_ge,
                            fill=NEG, base=-15, channel_multiplier=-16,
                            pattern=[[1, N]])
    # window mask on a [key r, query j] tile: visible iff r <= j <= r+63
    nc.gpsimd.memset(wmask[:], 0.0)
    nc.gpsimd.affine_select(out=wmask[:], in_=wmask[:], compare_op=AL.is_ge,
                            fill=NEG, base=0, channel_multiplier=-1,
                            pattern=[[1, 192]])
    nc.gpsimd.affine_select(out=wmask[:], in_=wmask[:], compare_op=AL.is_ge,
                            fill=NEG, base=63, channel_multiplier=1,
                            pattern=[[-1, 192]])

    nc.sync.dma_start(out=bs_sb[:], in_=bs_d.ap())
    nc.sync.dma_start(out=kcb_sb[:], in_=kcb_d.ap())
    nc.sync.dma_start(out=vcb_sb[:], in_=vcb_d.ap())
    nc.gpsimd.dma_start(out=wout_sb[:], in_=wout_d.ap())
    nc.gpsimd.dma_start(out=cwk_sb[:], in_=cwk_d.ap())
    nc.gpsimd.dma_start(out=cwv_sb[:], in_=cwv_d.ap())

    psP_cm = tc.tile_pool(name="psP", bufs=4, space="PSUM")
    psP = psP_cm.__enter__()
    sqp_cm = tc.tile_pool(name="sqp", bufs=2)
    sqp = sqp_cm.__enter__()

    # k-major phase A: RMS stats and the f0/f2 (q01/k01) projections
    # accumulate per input chunk, so the PE starts as soon as chunk 0 lands
    ps_s = [psP.tile([1, 512], F32, name=f"ps_s{ch}", bufs=1) for ch in range(2)]
    ppA = {(f, ch): psP.tile([128, 512], F32, name=f"pp{f}{ch}", bufs=1)
           for f in (0, 2) for ch in range(2)}
    for k in range(8):
        sq = sqp.tile([128, N], BF16, name="sq")
        if k % 2 == 0:
            nc.scalar.activation(sq[:], x_sb[:, k, :], AF.Square)
        else:
            nc.vector.tensor_tensor(sq[:], x_sb[:, k, :], x_sb[:, k, :], op=AL.mult)
        for ch in range(2):
            mm(ps_s[ch][:], ones_b[:, 0:1], sq[:, 512 * ch:512 * (ch + 1)],
               start=(k == 0), stop=(k == 7))
        for (f, ch), pp in ppA.items():
            sl = slice(512 * ch, 512 * (ch + 1))
            mm(pp[:], w_sb[:, k, 128 * f:128 * (f + 1)], x_sb[:, k, sl],
               start=(k == 0), stop=(k == 7))
    for ch in range(2):
        nc.scalar.activation(s_srt[0:1, 512 * ch:512 * (ch + 1)], ps_s[ch][:],
                             AF.Sqrt, bias=eps_sb[:], scale=1.0 / DIM)
    nc.vector.reciprocal_approx_fast(out=s_row[:], in_=s_srt[:])
    nc.gpsimd.partition_broadcast(s_bcast[:], s_row[:])
    for (f, ch), pp in ppA.items():
        sl = slice(512 * ch, 512 * (ch + 1))
        nc.vector.tensor_tensor(qk2[:, 0, f // 2, sl], pp[:, :],
                                s_bcast[:, sl], op=AL.mult)

    # qkv projection: f-tile layout [q01 | q23 | k01 | k23 | v01 | v23 | Ws]
    def proj_f(psP_, f_list, tag=None):
        for f in f_list:
            for ch in range(2):
                pp = psP_.tile([128, 512], F32, name="pp", tag=tag)
                sl = slice(512 * ch, 512 * (ch + 1))
                M = 128 if f < 6 else 3
                for k in range(8):
                    mm(pp[:M, :], w_sb[:, k, 128 * f:128 * f + M],
                       x_sb[:, k, sl], start=(k == 0), stop=(k == 7))
                if f < 6:
                    kind, hp = f // 2, f % 2
                    if kind < 2:
                        nc.vector.tensor_tensor(qk2[:, hp, kind, sl], pp[:, :],
                                                s_bcast[:, sl], op=AL.mult)
                    else:
                        nc.vector.tensor_tensor(v2[:, hp, sl], pp[:, :],
                                                s_bcast[:, sl], op=AL.mult)
                else:
                    nc.vector.tensor_tensor(w3r[:, sl], pp[:3, :],
                                            s_bcast[:3, sl], op=AL.mult)

    # phase A tail: v01 (f4) and Ws (f6) reuse the just-drained banks
    for f, names in ((4, ("pp00", "pp01")), (6, ("pp20", "pp21"))):
        for ch in range(2):
            pp = psP.tile([128, 512], F32, name=names[ch], bufs=1)
            sl = slice(512 * ch, 512 * (ch + 1))
            M = 128 if f < 6 else 3
            for k in range(8):
                mm(pp[:M, :], w_sb[:, k, 128 * f:128 * f + M],
                   x_sb[:, k, sl], start=(k == 0), stop=(k == 7))
            if f == 4:
                nc.vector.tensor_tensor(v2[:, 0, sl], pp[:, :],
                                        s_bcast[:, sl], op=AL.mult)
            else:
                nc.vector.tensor_tensor(w3r[:, sl], pp[:3, :],
                                        s_bcast[:3, sl], op=AL.mult)
    nc.scalar.activation(w3_sb[:], w3r[:], AF.Sigmoid, bias=bs_sb[:])
    nc.sync.dma_start(out=w1_row[:], in_=w3_sb[1:2, :])
    nc.sync.dma_start(out=q_od[:, 0, :], in_=qk2[64:128, 0, 0, :])
    nc.sync.dma_start(out=k_od[:, 0, :], in_=qk2[64:128, 0, 1, :])
    if DEBUG:
        nc.sync.dma_start(out=dbg["s"].ap(), in_=s_row[:])
        nc.sync.dma_start(out=dbg["w3"].ap(), in_=w3_sb[:])

    sqp_cm.__exit__(None, None, None)
    psP_cm.__exit__(None, None, None)

    # ----- stage 3-6: per-head attention --------------------------------
    att_cm = tc.tile_pool(name="att", bufs=1)
    att = att_cm.__enter__()
    ck_f2 = att.tile([128, 2, DH], BF16, name="ck_f2")
    cv_f2 = att.tile([128, 2, DH], BF16, name="cv_f2")
    ck_lo = att.tile([DH, 2, DH], BF16, name="ck_lo")
    cv_aug = att.tile([DH, 2, DH + 1], BF16, name="cv_aug")
    pc = att.tile([NB, 2, N], BF16, name="pc")
    pw = att.tile([128, 2, 8, 256], BF16, name="pw")
    vnat2 = att.tile([128, 2, 8, 130], BF16, name="vnat2")
    comb = att.tile([DH, HPC, N], BF16, name="comb")
    t1 = att.tile([DH, N], F32, name="t1")
    t2 = att.tile([DH, N], F32, name="t2")
    t3 = att.tile([DH, 896], F32, name="t3")
    dc64 = att.tile([65, N], F32, name="dc64")
    dw64 = att.tile([65, N], F32, name="dw64")
    dwbs = att.tile([65, 896], F32, name="dwbs")
    dcr = att.tile([1, N], F32, name="dcr")
    dwr = att.tile([1, N], F32, name="dwr")
    rcc = att.tile([1, N], F32, name="rcc")
    rcw = att.tile([1, N], F32, name="rcw")
    gcr = att.tile([1, N], BF16, name="gcr")
    gwr = att.tile([1, N], BF16, name="gwr")
    gc_b = att.tile([DH, N], BF16, name="gc_b")
    gw_b = att.tile([DH, N], BF16, name="gw_b")
    cmb, cmb_free = tc.tile([128, HPC, 2, N], BF16, name="cmb")

    # static pieces: ones columns for the AV denominators, zero pads for the
    # 192:256 query-span tails of the window probability tiles
    nc.gpsimd.memset(cv_aug[:, :, DH:DH + 1], 1.0)
    nc.gpsimd.memset(vnat2[:, :, :, 64:65], 1.0)
    nc.gpsimd.memset(vnat2[:, :, :, 129:130], 1.0)
    nc.gpsimd.memset(pw[:, :, :, 192:256], 0.0)

    dram_cm = tc.tile_pool(name="dram", bufs=1, space="DRAM")
    dram = dram_cm.__enter__()
    cc_in = dram.tile([HPC * DH, N], BF16, name="cc_in")
    cc_out = dram.tile([4 * HPC * DH, N], BF16, name="cc_out")

    def head_body(h, psA, psO):
        hp, par = h // 2, h % 2
        hb = par
        if par == 0:
            # V natural layout for both heads of the pair, via paired
            # [128,128] PE transposes; col 64 / 129 hold the ones columns.
            for g in range(8):
                ps_vt = psA.tile([128, 128], BF16, name="ps_vt", tag="psa")
                nc.tensor.transpose(ps_vt[:], v2[:, hp, 128 * g:128 * (g + 1)],
                                    ident[:, 0:128])
                nc.scalar.copy(vnat2[:, hp, g, 0:64], ps_vt[:, 0:64])
                nc.scalar.copy(vnat2[:, hp, g, 65:129], ps_vt[:, 64:128])

            # -- compression conv, both heads at once: contraction over the
            # pair-stacked 128 partitions with block-diagonal weights; the
            # moving operand reads block-strided columns of k/v in place.
            kmv = qk2[:, hp, 1, :].rearrange("p (c t) -> p t c", t=CB)
            vmv = v2[:, hp, :].rearrange("p (c t) -> p t c", t=CB)
            ps_ck = psA.tile([128, DH], F32, name="ps_ck", tag="psa")
            for t in range(CB):
                mm(ps_ck[:], cwk_sb[:, hp, t, :], kmv[:, t, :],
                   start=(t == 0), stop=(t == CB - 1))
            nc.scalar.activation(ck_f2[:, hp, :], ps_ck[:], AF.Identity,
                                 bias=kcb_sb[:, hp:hp + 1])
            ps_cv = psA.tile([128, DH], F32, name="ps_cv", tag="psa")
            for t in range(CB):
                mm(ps_cv[:], cwv_sb[:, hp, t, :], vmv[:, t, :],
                   start=(t == 0), stop=(t == CB - 1))
            nc.scalar.activation(cv_f2[:, hp, :], ps_cv[:], AF.Identity,
                                 bias=vcb_sb[:, hp:hp + 1])

        if par == 0:
            qT = qk2[0:64, hp, 0, :]
            kT = qk2[0:64, hp, 1, :]
            ckh = ck_f2[0:64, hp, :]
        else:
            qT = q_od[:, hp, :]
            kT = k_od[:, hp, :]
            nc.sync.dma_start(out=ck_lo[:, hp, :], in_=ck_f2[64:128, hp, :])
            ckh = ck_lo[:, hp, :]

        # cv to natural [block, dh] orientation (ones col already set)
        ps_cvt = psA.tile([DH, DH], BF16, name="ps_cvt", tag="psa")
        if par == 0:
            nc.tensor.transpose(ps_cvt[:], cv_f2[0:64, hp, :],
                                ident[0:64, 0:64])
        else:
            nc.tensor.transpose(ps_cvt[:], cv_f2[64:128, hp, :],
                                ident[64:128, 64:128])
        nc.scalar.copy(cv_aug[:, hb, 0:DH], ps_cvt[:])

        # -- compressed branch ------------------------------------------
        ps_oc = psO.tile([DH + 1, N], F32, name="ps_oc")
        for ch in range(2):
            sl = slice(512 * ch, 512 * (ch + 1))
            ps_sc = psA.tile([NB, 512], F32, name="ps_sc", tag="psa")
            mm(ps_sc[:], ckh, qT[:, sl], start=True, stop=True)
            nc.vector.tensor_tensor(ps_sc[:], ps_sc[:], cmask[:, sl], op=AL.add)
            nc.scalar.activation(pc[:, hb, sl], ps_sc[:], AF.Exp, scale=SCALE)
            mm(ps_oc[:, sl], cv_aug[:, hb, :], pc[:, hb, sl],
               start=True, stop=True)
        # compress gate computed right away: its whole staging / recip /
        # broadcast chain overlaps the window branch below
        nc.scalar.copy(dc64[64:65, :], ps_oc[64:65, :])
        nc.sync.dma_start(out=dcr[:], in_=dc64[64:65, :])
        nc.vector.reciprocal_approx_fast(out=rcc[:], in_=dcr[:])
        nc.vector.tensor_tensor(gcr[:], rcc[:], w3_sb[0:1, :], op=AL.mult)
        # tokens 0..14 see no compressed block: den==0 -> force gate to 0
        nc.vector.memset(gcr[0:1, 0:15], 0.0)
        nc.gpsimd.partition_broadcast(gc_b[:], gcr[0:1, :])

        # -- sliding window branch (first: its denominator chain overlaps
        # the compressed branch's PE work) ---------------------------------
        for g in range(8):
            nq = 192 if g < 7 else 128
            ps_sw = psA.tile([128, 192], F32, name="ps_sw", tag="psa")
            mm(ps_sw[:, :nq], kT[:, 128 * g:128 * (g + 1)],
               qT[:, 128 * g:128 * g + nq], start=True, stop=True)
            nc.vector.tensor_tensor(ps_sw[:, :nq], ps_sw[:, :nq],
                                    wmask[:, :nq], op=AL.add)
            nc.scalar.activation(pw[:, hb, g, 0:nq], ps_sw[:, :nq], AF.Exp,
                                 scale=SCALE)

        # window P@V: even key tiles cover query spans [256g0, 256g0+256),
        # odd tiles cover [128+256g, ...) in a second, 128-shifted psum.
        wsel = (lambda g: vnat2[:, hp, g, 0:65]) if par == 0 else \
               (lambda g: vnat2[:, hp, g, 65:130])
        ps_owA = psO.tile([DH + 1, N], F32, name="ps_owA")
        ps_owB = psO.tile([DH + 1, N], F32, name="ps_owB")
        for g in (0, 2, 4, 6):
            mm(ps_owA[:, 128 * g:128 * g + 256], wsel(g), pw[:, hb, g, 0:256],
               start=True, stop=True)
        for g in (1, 3, 5):
            mm(ps_owB[:, 128 * (g - 1):128 * (g - 1) + 256], wsel(g),
               pw[:, hb, g, 0:256], start=True, stop=True)
        mm(ps_owB[:, 768:896], wsel(7), pw[:, hb, 7, 0:128],
           start=True, stop=True)

        # compressed contribution first: frees ps_oc so the next head's
        # compress matmuls are not serialized behind this head's mix
        nc.vector.tensor_tensor(t1[:], ps_oc[0:DH, :], gc_b[:], op=AL.mult)
        # window denominator: A[0:128) plus the 128-shifted B row (rows at
        # psum partition 64), staged down to partition 0
        nc.scalar.copy(dw64[64:65, 0:128], ps_owA[64:65, 0:128])
        nc.scalar.copy(dwbs[64:65, :], ps_owB[64:65, 0:896])
        nc.vector.tensor_tensor(dw64[64:65, 128:N], ps_owA[64:65, 128:N],
                                dwbs[64:65, :], op=AL.add)
        nc.sync.dma_start(out=dwr[:], in_=dw64[64:65, :])
        nc.vector.reciprocal_approx_fast(out=rcw[:], in_=dwr[:])
        nc.vector.tensor_tensor(gwr[:], rcw[:], w1_row[:], op=AL.mult)
        nc.gpsimd.partition_broadcast(gw_b[:], gwr[0:1, :])
        nc.vector.tensor_tensor(t2[:], ps_owA[0:DH, :], gw_b[:], op=AL.mult)
        nc.vector.tensor_tensor(t3[:], ps_owB[0:DH, 0:896],
                                gw_b[:, 128:N], op=AL.mult)
        nc.vector.tensor_tensor(comb[:, h, :], t1[:], t2[:], op=AL.add)
        nc.vector.tensor_tensor(comb[:, h, 128:N], comb[:, h, 128:N],
                                t3[:], op=AL.add)

        if DEBUG and h == 0:
            nc.sync.dma_start(out=dbg["ck"].ap(), in_=ck_f2[:, hp, :])
            nc.sync.dma_start(out=dbg["cva"].ap(), in_=cv_aug[:, hb, :])
        if DEBUG and h == 1:
            nc.sync.dma_start(out=dbg["pc"].ap(), in_=pc[:])
            nc.sync.dma_start(out=dbg["pw"].ap(), in_=pw[:])
            nc.sync.dma_start(out=dbg["vnat"].ap(), in_=vnat2[:])

        # -- per-head AllGather chunk ------------------------------------
        nc.sync.dma_start(out=cc_in[64 * h:64 * (h + 1), :], in_=comb[:, h, :])
        nc.gpsimd.collective_compute(
            "AllGather", AL.bypass, replica_groups=GROUPS,
            ins=[cc_in[64 * h:64 * (h + 1), :].opt()],
            outs=[cc_out[256 * h:256 * (h + 1), :].opt()])

    # pair 0 attention; pair-1 projection is emitted right after head 1's PE
    # work out of the still-open scratch pool, so the PE never drains while
    # the pair-0 gathers' cross-core skew is absorbed; then pair 1.
    for phase in range(2):
        psO_cm = tc.tile_pool(name=f"psO{phase}", bufs=1, space="PSUM")
        psO = psO_cm.__enter__()
        psA_cm = tc.tile_pool(name=f"psA{phase}", bufs=2, space="PSUM")
        psA = psA_cm.__enter__()
        for h in (2 * phase, 2 * phase + 1):
            head_body(h, psA, psO)
        psA_cm.__exit__(None, None, None)
        if phase == 0:
            # pair-1 projection in the banks the scratch pool just freed;
            # psO stays open for head 1's still-pending mix reads
            psP2_cm = tc.tile_pool(name="psP2", bufs=2, space="PSUM")
            psP2 = psP2_cm.__enter__()
            proj_f(psP2, (1, 3, 5))
            nc.sync.dma_start(out=q_od[:, 1, :], in_=qk2[64:128, 1, 0, :])
            nc.sync.dma_start(out=k_od[:, 1, :], in_=qk2[64:128, 1, 1, :])
            psP2_cm.__exit__(None, None, None)
        psO_cm.__exit__(None, None, None)

    if DEBUG:
        nc.sync.dma_start(out=dbg["comb"].ap(), in_=comb[:])
    if DEBUG:
        nc.sync.dma_start(out=dbg["qk"].ap(), in_=qk2[:])
        nc.sync.dma_start(out=dbg["v2"].ap(), in_=v2[:])

    # ----- stage 7: output projection over gathered head chunks ----------
    for h in range(HPC):
        for s in range(2):
            nc.gpsimd.dma_start(
                out=cmb[:, h, s, :],
                in_=cc_out[256 * h + 128 * s:256 * h + 128 * (s + 1), :])
    if DEBUG:
        nc.sync.dma_start(out=dbg["cmb"].ap(), in_=cmb[:])

    psW_cm = tc.tile_pool(name="psW", bufs=4, space="PSUM")
    psW = psW_cm.__enter__()
    po = {}
    for m in range(2):
        for ch in range(2):
            po[(m, ch)] = psW.tile([128, 512], F32, name=f"po{m}{ch}", bufs=1)
    for h in range(HPC):
        for s in range(2):
            for m in range(2):
                for ch in range(2):
                    sl = slice(512 * ch, 512 * (ch + 1))
                    mm(po[(m, ch)][:], wout_sb[:, h, s, 128 * m:128 * (m + 1)],
                       cmb[:, h, s, sl], start=(h == 0 and s == 0),
                       stop=(h == HPC - 1 and s == 1))
    outT_sb, outT_sb_free = tc.tile([128, 2, N], F32, name="outT_sb")
    for m in range(2):
        for ch in range(2):
            sl = slice(512 * ch, 512 * (ch + 1))
            nc.scalar.copy(outT_sb[:, m, sl], po[(m, ch)][:])
            qe = nc.sync if ch == 0 else nc.gpsimd
            qe.dma_start(out=outT_d.ap()[128 * m:128 * (m + 1), sl],
                         in_=outT_sb[:, m, sl])

    psW_cm.__exit__(None, None, None)
    outT_sb_free()
    cmb_free()
    dram_cm.__exit__(None, None, None)
    att_cm.__exit__(None, None, None)
    w_free()
    x_free()
    k_od_free()
    q_od_free()
    v2_free()
    qk2_free()
    const_cm.__exit__(None, None, None)


# --------------------------------------------------------------------------
_CACHE: dict = {}


def _get_nc() -> bass.Bass:
    if "nc" not in _CACHE:
        _CACHE["nc"] = build_program()
    return _CACHE["nc"]


def _prep_core(c: int, inputs: dict) -> dict:
    b, r = c // 4, c % 4
    hs = HPC * r
    f32, f64 = np.float32, np.float64
    inp = np.asarray(inputs["inp"], f32)
    rms_w = np.asarray(inputs["rms_w"], f32)
    Wqkv = np.asarray(inputs["Wqkv"], f32)
    k_pos = np.asarray(inputs["k_pos"], f32)
    v_pos = np.asarray(inputs["v_pos"], f32)
    k_cw = np.asarray(inputs["k_cw"], f32)
    k_cb = np.asarray(inputs["k_cb"], f32)
    v_cw = np.asarray(inputs["v_cw"], f32)
    v_cb = np.asarray(inputs["v_cb"], f32)
    Ws = np.asarray(inputs["Ws"], f32)
    bs = np.asarray(inputs["bs"], f32)
    Wout = np.asarray(inputs["Wout"], f32)

    # rms_w folded into the projection weights
    cols = [Wqkv[:, p * H * DH + hs * DH: p * H * DH + (hs + HPC) * DH]
            for p in range(3)]
    w_all = np.concatenate(cols + [Ws], axis=1) * rms_w[:, None].astype(f64)
    w_all = np.ascontiguousarray(w_all).astype(NPB)

    # conv weights: per head pair, block-diagonal [64*ha + i, pair, t, 64*hb + o]
    def conv_pack(cw):
        t = np.zeros((128, 2, CB, 128), f32)
        for pair in range(2):
            for half in range(2):
                blk = cw[hs + 2 * pair + half].transpose(1, 2, 0)  # [i, t, o]
                t[64 * half:64 * half + 64, pair, :,
                  64 * half:64 * half + 64] = blk
        return np.ascontiguousarray(t).astype(NPB)

    # conv(pos) folded into the conv bias: bias_eff packed [64*hb + o, pair]
    def bias_fold(cb, cw, pos):
        cp = np.einsum("hoit,hti->ho", cw[hs:hs + HPC].astype(f64),
                       pos[hs:hs + HPC].astype(f64))
        be = cb[hs:hs + HPC] + cp                        # [4, o]
        out = np.zeros((128, 2), f32)
        for pair in range(2):
            for half in range(2):
                out[64 * half:64 * (half + 1), pair] = be[2 * pair + half]
        return np.ascontiguousarray(out)

    # output projection slabs: chunk h delivers global heads (8s+h, 8s+4+h)
    woutS = np.zeros((128, HPC, 2, 256), f32)
    for h in range(HPC):
        for s in range(2):
            ga, gb = 8 * s + h, 8 * s + 4 + h
            woutS[0:64, h, s, :] = Wout[64 * ga:64 * ga + 64,
                                        256 * r:256 * (r + 1)]
            woutS[64:128, h, s, :] = Wout[64 * gb:64 * gb + 64,
                                          256 * r:256 * (r + 1)]

    return {
        "inpT": np.ascontiguousarray(inp[b].T).astype(NPB),
        "w_all": w_all,
        "cw_k": conv_pack(k_cw),
        "cw_v": conv_pack(v_cw),
        "kcb": bias_fold(k_cb, k_cw, k_pos),
        "vcb": bias_fold(v_cb, v_cw, v_pos),
        "bs_t": np.ascontiguousarray(bs[:, None]),
        "woutS": woutS.astype(NPB),
        "ones_c": np.ones((128, 8), NPB),
        "ident_c": np.eye(128, dtype=NPB),
    }


def kernel(**inputs) -> np.ndarray:
    nc = _get_nc()
    in_maps = [_prep_core(c, inputs) for c in range(NCORES)]
    res = run_bass_kernel_spmd(nc, in_maps, list(range(NCORES)))
    out = np.zeros((B, N, DIM), np.float32)
    for c in range(NCORES):
        b, r = c // 4, c % 4
        out[b, :, 256 * r:256 * (r + 1)] = res.results[c]["outT"].T
    return out


# revision 54
# speedup vs baseline: 1.1361x; 1.1361x over previous
"""Trainium2 Bass kernel for nn_Attention_41686952575399 (sparse attention).

Sharding: data-parallel over batch (2 groups of 4 cores) x tensor-parallel over
heads (4 heads per core). Per-head device-side AllGather chunks (bf16) overlap
with the next head's compute; each core then computes a 256-wide dout slice of
the output projection for all tokens of its batch element.

vs the fp32r baseline:
  - bf16 datapath for all matmul operands (psum accumulation stays fp32);
    halves HBM loads and collective bytes.
  - rms_w / conv positional embedding / conv bias folded on the host.
  - compression conv as 8 accumulating matmuls with (dh, token-parity) packed
    128-partition contraction instead of 16 64-contraction matmuls.
  - window P@V via 8 matmuls into two 256-col-span psum accumulators (even /
    odd key tiles, odd spans shifted 128 cols) instead of 15 128-col matmuls.
  - paired PE transposes produce V in natural layout for two heads at once.
  - reciprocal_approx_fast for softmax denominators and RMS norm.
  - per-head AllGather issued right after each head's mix -> only the last
    ~quarter of the collective is exposed.
"""
import os
import sys

sys.path.insert(0, "/opt/trn_rl_repo")

DEBUG = os.environ.get("BASSK_DEBUG") == "1"

import numpy as np
import ml_dtypes

from concourse import bacc, bass, mybir, tile
from concourse.bass_utils import run_bass_kernel_spmd

B, N, DIM = 2, 1024, 1024
H, DH = 16, 64
WIN, CB = 64, 16
NB = N // CB               # 64 compressed blocks
HPC = 4                    # heads per core
NCORES = 8
GROUPS = [[0, 1, 2, 3], [4, 5, 6, 7]]
F32 = mybir.dt.float32
BF16 = mybir.dt.bfloat16
NEG = -1e30
EPS = float(np.finfo(np.float32).eps)
SCALE = float(DH ** -0.5)
NF = 3 * HPC * DH + 3      # 771 projection output features (q,k,v slices + Ws)
NPB = ml_dtypes.bfloat16

AL = mybir.AluOpType
AF = mybir.ActivationFunctionType


def build_program() -> bass.Bass:
    nc = bacc.Bacc("TRN2", target_bir_lowering=False, debug=False,
                   num_devices=NCORES)

    inpT_d = nc.dram_tensor("inpT", [DIM, N], BF16, kind="ExternalInput")
    wall_d = nc.dram_tensor("w_all", [DIM, NF], BF16, kind="ExternalInput")
    cwk_d = nc.dram_tensor("cw_k", [128, 2, CB, 128], BF16, kind="ExternalInput")
    cwv_d = nc.dram_tensor("cw_v", [128, 2, CB, 128], BF16, kind="ExternalInput")
    kcb_d = nc.dram_tensor("kcb", [128, 2], F32, kind="ExternalInput")
    vcb_d = nc.dram_tensor("vcb", [128, 2], F32, kind="ExternalInput")
    bs_d = nc.dram_tensor("bs_t", [3, 1], F32, kind="ExternalInput")
    wout_d = nc.dram_tensor("woutS", [128, HPC, 2, 256], BF16, kind="ExternalInput")
    ones_d = nc.dram_tensor("ones_c", [128, 8], BF16, kind="ExternalInput")
    ident_d = nc.dram_tensor("ident_c", [128, 128], BF16, kind="ExternalInput")
    outT_d = nc.dram_tensor("outT", [256, N], F32, kind="ExternalOutput")
    dbg = {}
    if DEBUG:
        dbg["s"] = nc.dram_tensor("dbg_s", [1, N], F32, kind="ExternalOutput")
        dbg["w3"] = nc.dram_tensor("dbg_w3", [3, N], F32, kind="ExternalOutput")
        dbg["qk"] = nc.dram_tensor("dbg_qk", [128, 2, 2, N], BF16,
                                   kind="ExternalOutput")
        dbg["v2"] = nc.dram_tensor("dbg_v2", [128, 2, N], BF16,
                                   kind="ExternalOutput")
        dbg["ck"] = nc.dram_tensor("dbg_ck", [128, DH], BF16, kind="ExternalOutput")
        dbg["cva"] = nc.dram_tensor("dbg_cva", [DH, DH + 1], BF16,
                                    kind="ExternalOutput")
        dbg["pc"] = nc.dram_tensor("dbg_pc", [NB, 2, N], BF16, kind="ExternalOutput")
        dbg["pw"] = nc.dram_tensor("dbg_pw", [128, 2, 8, 256], BF16,
                                   kind="ExternalOutput")
        dbg["vnat"] = nc.dram_tensor("dbg_vnat", [128, 2, 8, 130], BF16,
                                     kind="ExternalOutput")
        dbg["den"] = nc.dram_tensor("dbg_den", [2, N], F32, kind="ExternalOutput")
        dbg["denr"] = nc.dram_tensor("dbg_denr", [2, N], F32, kind="ExternalOutput")
        dbg["comb"] = nc.dram_tensor("dbg_comb", [DH, HPC, N], BF16,
                                     kind="ExternalOutput")
        dbg["cmb"] = nc.dram_tensor("dbg_cmb", [128, HPC, 2, N], BF16,
                                    kind="ExternalOutput")

    with tile.TileContext(nc) as tc:
        _body(nc, tc, inpT_d, wall_d, cwk_d, cwv_d, kcb_d, vcb_d, bs_d,
              wout_d, ones_d, ident_d, outT_d, dbg)
    nc.compile()
    return nc


def _body(nc, tc, inpT_d, wall_d, cwk_d, cwv_d, kcb_d, vcb_d, bs_d,
          wout_d, ones_d, ident_d, outT_d, dbg):
    mm = nc.tensor.matmul

    # ----- long-lived constants -----------------------------------------
    const_cm = tc.tile_pool(name="const", bufs=1)
    const = const_cm.__enter__()
    ones_b = const.tile([128, 8], BF16, name="ones_b")
    ident = const.tile([128, 128], BF16, name="ident")
    cmask = const.tile([NB, N], F32, name="cmask")
    wmask = const.tile([128, 192], F32, name="wmask")
    bs_sb = const.tile([3, 1], F32, name="bs_sb")
    kcb_sb = const.tile([128, 2], F32, name="kcb_sb")
    vcb_sb = const.tile([128, 2], F32, name="vcb_sb")
    eps_sb = const.tile([1, 1], F32, name="eps_sb")
    s_srt = const.tile([1, N], F32, name="s_srt")
    s_row = const.tile([1, N], F32, name="s_row")
    s_bcast = const.tile([128, N], F32, name="s_bcast")
    w3r = const.tile([3, N], F32, name="w3r")
    w3_sb = const.tile([3, N], F32, name="w3_sb")
    w1_row = const.tile([1, N], F32, name="w1_row")
    wout_sb = const.tile([128, HPC, 2, 256], BF16, name="wout_sb")
    cwk_sb = const.tile([128, 2, CB, 128], BF16, name="cwk_sb")
    cwv_sb = const.tile([128, 2, CB, 128], BF16, name="cwv_sb")

    # ----- stage 1+2: RMS stats + fused qkv/Ws projection ---------------
    # qk2: q/k with even head of the pair on partitions 0-63, odd on 64-127.
    qk2, qk2_free = tc.tile([128, 2, 2, N], BF16, name="qk2")
    v2, v2_free = tc.tile([128, 2, N], BF16, name="v2")
    q_od, q_od_free = tc.tile([DH, 2, N], BF16, name="q_od")
    k_od, k_od_free = tc.tile([DH, 2, N], BF16, name="k_od")
    x_sb, x_free = tc.tile([128, 8, N], BF16, name="x_sb")
    w_sb, w_free = tc.tile([128, 8, NF], BF16, name="w_sb")

    # x/w loads first (they gate all compute), split across two queues
    for k in range(8):
        nc.gpsimd.dma_start(out=x_sb[:, k, :], in_=inpT_d.ap()[128 * k:128 * (k + 1), :])
        nc.sync.dma_start(out=w_sb[:, k, :], in_=wall_d.ap()[128 * k:128 * (k + 1), :])
    nc.gpsimd.dma_start(out=ones_b[:], in_=ones_d.ap())
    nc.gpsimd.dma_start(out=ident[:], in_=ident_d.ap())
    nc.gpsimd.memset(eps_sb[:], EPS)
    # compressed-block causal mask: block c visible to token t iff t >= 16c+15
    nc.gpsimd.memset(cmask[:], 0.0)
    nc.gpsimd.affine_select(out=cmask[:], in_=cmask[:], compare_op=AL.is_ge,
                            fill=NEG, base=-15, channel_multiplier=-16,
                            pattern=[[1, N]])
    # window mask on a [key r, query j] tile: visible iff r <= j <= r+63
    nc.gpsimd.memset(wmask[:], 0.0)
    nc.gpsimd.affine_select(out=wmask[:], in_=wmask[:], compare_op=AL.is_ge,
                            fill=NEG, base=0, channel_multiplier=-1,
                            pattern=[[1, 192]])
    nc.gpsimd.affine_select(out=wmask[:], in_=wmask[:], compare_op=AL.is_ge,
                            fill=NEG, base=63, channel_multiplier=1,
                            pattern=[[-1, 192]])

    nc.sync.dma_start(out=bs_sb[:], in_=bs_d.ap())
    nc.sync.dma_start(out=kcb_sb[:], in_=kcb_d.ap())
    nc.sync.dma_start(out=vcb_sb[:], in_=vcb_d.ap())
    nc.gpsimd.dma_start(out=wout_sb[:], in_=wout_d.ap())
    nc.gpsimd.dma_start(out=cwk_sb[:], in_=cwk_d.ap())
    nc.gpsimd.dma_start(out=cwv_sb[:], in_=cwv_d.ap())

    psP_cm = tc.tile_pool(name="psP", bufs=4, space="PSUM")
    psP = psP_cm.__enter__()
    sqp_cm = tc.tile_pool(name="sqp", bufs=2)
    sqp = sqp_cm.__enter__()

    # k-major phase A: RMS stats and the f0/f2 (q01/k01) projections
    # accumulate per input chunk, so the PE starts as soon as chunk 0 lands
    ps_s = [psP.tile([1, 512], F32, name=f"ps_s{ch}", bufs=1) for ch in range(2)]
    ppA = {(f, ch): psP.tile([128, 512], F32, name=f"pp{f}{ch}", bufs=1)
           for f in (0, 2) for ch in range(2)}
    for k in range(8):
        sq = sqp.tile([128, N], BF16, name="sq")
        if k % 2 == 0:
            nc.scalar.activation(sq[:], x_sb[:, k, :], AF.Square)
        else:
            nc.vector.tensor_tensor(sq[:], x_sb[:, k, :], x_sb[:, k, :], op=AL.mult)
        for ch in range(2):
            mm(ps_s[ch][:], ones_b[:, 0:1], sq[:, 512 * ch:512 * (ch + 1)],
               start=(k == 0), stop=(k == 7))
        for (f, ch), pp in ppA.items():
            sl = slice(512 * ch, 512 * (ch + 1))
            mm(pp[:], w_sb[:, k, 128 * f:128 * (f + 1)], x_sb[:, k, sl],
               start=(k == 0), stop=(k == 7))
    for ch in range(2):
        nc.scalar.activation(s_srt[0:1, 512 * ch:512 * (ch + 1)], ps_s[ch][:],
                             AF.Sqrt, bias=eps_sb[:], scale=1.0 / DIM)
    nc.vector.reciprocal_approx_fast(out=s_row[:], in_=s_srt[:])
    nc.gpsimd.partition_broadcast(s_bcast[:], s_row[:])
    for (f, ch), pp in ppA.items():
        sl = slice(512 * ch, 512 * (ch + 1))
        nc.vector.tensor_tensor(qk2[:, 0, f // 2, sl], pp[:, :],
                                s_bcast[:, sl], op=AL.mult)

    # qkv projection: f-tile layout [q01 | q23 | k01 | k23 | v01 | v23 | Ws]
    def proj_f(psP_, f_list, tag=None):
        for f in f_list:
            for ch in range(2):
                pp = psP_.tile([128, 512], F32, name="pp", tag=tag)
                sl = slice(512 * ch, 512 * (ch + 1))
                M = 128 if f < 6 else 3
                for k in range(8):
                    mm(pp[:M, :], w_sb[:, k, 128 * f:128 * f + M],
                       x_sb[:, k, sl], start=(k == 0), stop=(k == 7))
                if f < 6:
                    kind, hp = f // 2, f % 2
                    if kind < 2:
                        nc.vector.tensor_tensor(qk2[:, hp, kind, sl], pp[:, :],
                                                s_bcast[:, sl], op=AL.mult)
                    else:
                        nc.vector.tensor_tensor(v2[:, hp, sl], pp[:, :],
                                                s_bcast[:, sl], op=AL.mult)
                else:
                    nc.vector.tensor_tensor(w3r[:, sl], pp[:3, :],
                                            s_bcast[:3, sl], op=AL.mult)

    # phase A tail: v01 (f4) and Ws (f6) reuse the just-drained banks
    for f, names in ((4, ("pp00", "pp01")), (6, ("pp20", "pp21"))):
        for ch in range(2):
            pp = psP.tile([128, 512], F32, name=names[ch], bufs=1)
            sl = slice(512 * ch, 512 * (ch + 1))
            M = 128 if f < 6 else 3
            for k in range(8):
                mm(pp[:M, :], w_sb[:, k, 128 * f:128 * f + M],
                   x_sb[:, k, sl], start=(k == 0), stop=(k == 7))
            if f == 4:
                nc.vector.tensor_tensor(v2[:, 0, sl], pp[:, :],
                                        s_bcast[:, sl], op=AL.mult)
            else:
                nc.vector.tensor_tensor(w3r[:, sl], pp[:3, :],
                                        s_bcast[:3, sl], op=AL.mult)
    nc.scalar.activation(w3_sb[:], w3r[:], AF.Sigmoid, bias=bs_sb[:])
    nc.sync.dma_start(out=w1_row[:], in_=w3_sb[1:2, :])
    nc.sync.dma_start(out=q_od[:, 0, :], in_=qk2[64:128, 0, 0, :])
    nc.sync.dma_start(out=k_od[:, 0, :], in_=qk2[64:128, 0, 1, :])
    if DEBUG:
        nc.sync.dma_start(out=dbg["s"].ap(), in_=s_row[:])
        nc.sync.dma_start(out=dbg["w3"].ap(), in_=w3_sb[:])

    sqp_cm.__exit__(None, None, None)
    psP_cm.__exit__(None, None, None)

    # ----- stage 3-6: per-head attention --------------------------------
    att_cm = tc.tile_pool(name="att", bufs=1)
    att = att_cm.__enter__()
    ck_f2 = att.tile([128, 2, DH], BF16, name="ck_f2")
    cv_f2 = att.tile([128, 2, DH], BF16, name="cv_f2")
    ck_lo = att.tile([DH, 2, DH], BF16, name="ck_lo")
    cv_aug = att.tile([DH, 2, DH + 1], BF16, name="cv_aug")
    pc = att.tile([NB, 2, N], BF16, name="pc")
    pw = att.tile([128, 2, 8, 256], BF16, name="pw")
    vnat2 = att.tile([128, 2, 8, 130], BF16, name="vnat2")
    comb = att.tile([DH, HPC, N], BF16, name="comb")
    t1 = att.tile([DH, N], F32, name="t1")
    t2 = att.tile([DH, N], F32, name="t2")
    t3 = att.tile([DH, 896], F32, name="t3")
    dc64 = att.tile([65, N], F32, name="dc64")
    dw64 = att.tile([65, N], F32, name="dw64")
    dwbs = att.tile([65, 896], F32, name="dwbs")
    d2 = att.tile([2, N], F32, name="d2")
    r2 = att.tile([2, N], F32, name="r2")
    g2b = att.tile([2, N], BF16, name="g2b")
    gwrow_b = att.tile([1, N], BF16, name="gwrow_b")
    gc_b = att.tile([DH, N], BF16, name="gc_b")
    gw_b = att.tile([DH, N], BF16, name="gw_b")
    cmb, cmb_free = tc.tile([128, HPC, 2, N], BF16, name="cmb")

    # static pieces: ones columns for the AV denominators, zero pads for the
    # 192:256 query-span tails of the window probability tiles
    nc.gpsimd.memset(cv_aug[:, :, DH:DH + 1], 1.0)
    nc.gpsimd.memset(vnat2[:, :, :, 64:65], 1.0)
    nc.gpsimd.memset(vnat2[:, :, :, 129:130], 1.0)
    nc.gpsimd.memset(pw[:, :, :, 192:256], 0.0)

    dram_cm = tc.tile_pool(name="dram", bufs=1, space="DRAM")
    dram = dram_cm.__enter__()
    cc_in = dram.tile([HPC * DH, N], BF16, name="cc_in")
    cc_out = dram.tile([4 * HPC * DH, N], BF16, name="cc_out")

    def head_body(h, psA, psO):
        hp, par = h // 2, h % 2
        hb = par
        if par == 0:
            # V natural layout for both heads of the pair, via paired
            # [128,128] PE transposes; col 64 / 129 hold the ones columns.
            for g in range(8):
                ps_vt = psA.tile([128, 128], BF16, name="ps_vt", tag="psa")
                nc.tensor.transpose(ps_vt[:], v2[:, hp, 128 * g:128 * (g + 1)],
                                    ident[:, 0:128])
                nc.scalar.copy(vnat2[:, hp, g, 0:64], ps_vt[:, 0:64])
                nc.scalar.copy(vnat2[:, hp, g, 65:129], ps_vt[:, 64:128])

            # -- compression conv, both heads at once: contraction over the
            # pair-stacked 128 partitions with block-diagonal weights; the
            # moving operand reads block-strided columns of k/v in place.
            kmv = qk2[:, hp, 1, :].rearrange("p (c t) -> p t c", t=CB)
            vmv = v2[:, hp, :].rearrange("p (c t) -> p t c", t=CB)
            ps_ck = psA.tile([128, DH], F32, name="ps_ck", tag="psa")
            for t in range(CB):
                mm(ps_ck[:], cwk_sb[:, hp, t, :], kmv[:, t, :],
                   start=(t == 0), stop=(t == CB - 1))
            nc.scalar.activation(ck_f2[:, hp, :], ps_ck[:], AF.Identity,
                                 bias=kcb_sb[:, hp:hp + 1])
            ps_cv = psA.tile([128, DH], F32, name="ps_cv", tag="psa")
            for t in range(CB):
                mm(ps_cv[:], cwv_sb[:, hp, t, :], vmv[:, t, :],
                   start=(t == 0), stop=(t == CB - 1))
            nc.scalar.activation(cv_f2[:, hp, :], ps_cv[:], AF.Identity,
                                 bias=vcb_sb[:, hp:hp + 1])

        if par == 0:
            qT = qk2[0:64, hp, 0, :]
            kT = qk2[0:64, hp, 1, :]
            ckh = ck_f2[0:64, hp, :]
        else:
            qT = q_od[:, hp, :]
            kT = k_od[:, hp, :]
            nc.sync.dma_start(out=ck_lo[:, hp, :], in_=ck_f2[64:128, hp, :])
            ckh = ck_lo[:, hp, :]

        # cv to natural [block, dh] orientation (ones col already set)
        ps_cvt = psA.tile([DH, DH], BF16, name="ps_cvt", tag="psa")
        if par == 0:
            nc.tensor.transpose(ps_cvt[:], cv_f2[0:64, hp, :],
                                ident[0:64, 0:64])
        else:
            nc.tensor.transpose(ps_cvt[:], cv_f2[64:128, hp, :],
                                ident[64:128, 64:128])
        nc.scalar.copy(cv_aug[:, hb, 0:DH], ps_cvt[:])

        # -- compressed branch ------------------------------------------
        ps_oc = psO.tile([DH + 1, N], F32, name="ps_oc")
        for ch in range(2):
            sl = slice(512 * ch, 512 * (ch + 1))
            ps_sc = psA.tile([NB, 512], F32, name="ps_sc", tag="psa")
            mm(ps_sc[:], ckh, qT[:, sl], start=True, stop=True)
            nc.vector.tensor_tensor(ps_sc[:], ps_sc[:], cmask[:, sl], op=AL.add)
            nc.scalar.activation(pc[:, hb, sl], ps_sc[:], AF.Exp, scale=SCALE)
            mm(ps_oc[:, sl], cv_aug[:, hb, :], pc[:, hb, sl],
               start=True, stop=True)
        # compress denominator staged right away; the window branch below
        # overlaps the copy + shift latency
        nc.scalar.copy(dc64[64:65, :], ps_oc[64:65, :])
        nc.sync.dma_start(out=d2[0:1, :], in_=dc64[64:65, :])

        # -- sliding window branch (first: its denominator chain overlaps
        # the compressed branch's PE work) ---------------------------------
        for g in range(8):
            nq = 192 if g < 7 else 128
            ps_sw = psA.tile([128, 192], F32, name="ps_sw", tag="psa")
            mm(ps_sw[:, :nq], kT[:, 128 * g:128 * (g + 1)],
               qT[:, 128 * g:128 * g + nq], start=True, stop=True)
            nc.vector.tensor_tensor(ps_sw[:, :nq], ps_sw[:, :nq],
                                    wmask[:, :nq], op=AL.add)
            nc.scalar.activation(pw[:, hb, g, 0:nq], ps_sw[:, :nq], AF.Exp,
                                 scale=SCALE)

        # window P@V: even key tiles cover query spans [256g0, 256g0+256),
        # odd tiles cover [128+256g, ...) in a second, 128-shifted psum.
        wsel = (lambda g: vnat2[:, hp, g, 0:65]) if par == 0 else \
               (lambda g: vnat2[:, hp, g, 65:130])
        ps_owA = psO.tile([DH + 1, N], F32, name="ps_owA")
        ps_owB = psO.tile([DH + 1, N], F32, name="ps_owB")
        for g in (0, 2, 4, 6):
            mm(ps_owA[:, 128 * g:128 * g + 256], wsel(g), pw[:, hb, g, 0:256],
               start=True, stop=True)
        for g in (1, 3, 5):
            mm(ps_owB[:, 128 * (g - 1):128 * (g - 1) + 256], wsel(g),
               pw[:, hb, g, 0:256], start=True, stop=True)
        mm(ps_owB[:, 768:896], wsel(7), pw[:, hb, 7, 0:128],
           start=True, stop=True)

        # window denominator: A[0:128) plus the 128-shifted B row (rows at
        # psum partition 64), staged down to partitions 0/1, one recip
        nc.scalar.copy(dw64[64:65, 0:128], ps_owA[64:65, 0:128])
        nc.scalar.copy(dwbs[64:65, :], ps_owB[64:65, 0:896])
        nc.vector.tensor_tensor(dw64[64:65, 128:N], ps_owA[64:65, 128:N],
                                dwbs[64:65, :], op=AL.add)
        nc.sync.dma_start(out=d2[1:2, :], in_=dw64[64:65, :])
        nc.vector.reciprocal_approx_fast(out=r2[:], in_=d2[:])
        nc.vector.tensor_tensor(g2b[:], r2[:], w3_sb[0:2, :], op=AL.mult)
        # tokens 0..14 see no compressed block: den==0 -> force gate to 0
        nc.vector.memset(g2b[0:1, 0:15], 0.0)
        nc.sync.dma_start(out=gwrow_b[:], in_=g2b[1:2, :])
        nc.gpsimd.partition_broadcast(gc_b[:], g2b[0:1, :])
        nc.gpsimd.partition_broadcast(gw_b[:], gwrow_b[0:1, :])
        nc.vector.tensor_tensor(t2[:], ps_owA[0:DH, :], gw_b[:], op=AL.mult)
        nc.vector.tensor_tensor(t3[:], ps_owB[0:DH, 0:896],
                                gw_b[:, 128:N], op=AL.mult)
        nc.vector.tensor_tensor(t1[:], ps_oc[0:DH, :], gc_b[:], op=AL.mult)
        nc.vector.tensor_tensor(comb[:, h, :], t1[:], t2[:], op=AL.add)
        nc.vector.tensor_tensor(comb[:, h, 128:N], comb[:, h, 128:N],
                                t3[:], op=AL.add)

        if DEBUG and h == 0:
            nc.sync.dma_start(out=dbg["ck"].ap(), in_=ck_f2[:, hp, :])
            nc.sync.dma_start(out=dbg["cva"].ap(), in_=cv_aug[:, hb, :])
        if DEBUG and h == 1:
            nc.sync.dma_start(out=dbg["pc"].ap(), in_=pc[:])
            nc.sync.dma_start(out=dbg["pw"].ap(), in_=pw[:])
            nc.sync.dma_start(out=dbg["vnat"].ap(), in_=vnat2[:])

        # -- per-head AllGather chunk ------------------------------------
        nc.sync.dma_start(out=cc_in[64 * h:64 * (h + 1), :], in_=comb[:, h, :])
        nc.gpsimd.collective_compute(
            "AllGather", AL.bypass, replica_groups=GROUPS,
            ins=[cc_in[64 * h:64 * (h + 1), :].opt()],
            outs=[cc_out[256 * h:256 * (h + 1), :].opt()])

    # pair 0 attention; pair-1 projection is emitted right after head 1's PE
    # work out of the still-open scratch pool, so the PE never drains while
    # the pair-0 gathers' cross-core skew is absorbed; then pair 1.
    for phase in range(2):
        psO_cm = tc.tile_pool(name=f"psO{phase}", bufs=1, space="PSUM")
        psO = psO_cm.__enter__()
        psA_cm = tc.tile_pool(name=f"psA{phase}", bufs=2, space="PSUM")
        psA = psA_cm.__enter__()
        for h in (2 * phase, 2 * phase + 1):
            head_body(h, psA, psO)
        psA_cm.__exit__(None, None, None)
        if phase == 0:
            # pair-1 projection in the banks the scratch pool just freed;
            # psO stays open for head 1's still-pending mix reads
            psP2_cm = tc.tile_pool(name="psP2", bufs=2, space="PSUM")
            psP2 = psP2_cm.__enter__()
            proj_f(psP2, (1, 3, 5))
            nc.sync.dma_start(out=q_od[:, 1, :], in_=qk2[64:128, 1, 0, :])
            nc.sync.dma_start(out=k_od[:, 1, :], in_=qk2[64:128, 1, 1, :])
            psP2_cm.__exit__(None, None, None)
        psO_cm.__exit__(None, None, None)

    if DEBUG:
        nc.sync.dma_start(out=dbg["comb"].ap(), in_=comb[:])
    if DEBUG:
        nc.sync.dma_start(out=dbg["qk"].ap(), in_=qk2[:])
        nc.sync.dma_start(out=dbg["v2"].ap(), in_=v2[:])

    # ----- stage 7: output projection over gathered head chunks ----------
    for h in range(HPC):
        for s in range(2):
            nc.gpsimd.dma_start(
                out=cmb[:, h, s, :],
                in_=cc_out[256 * h + 128 * s:256 * h + 128 * (s + 1), :])
    if DEBUG:
        nc.sync.dma_start(out=dbg["cmb"].ap(), in_=cmb[:])

    psW_cm = tc.tile_pool(name="psW", bufs=4, space="PSUM")
    psW = psW_cm.__enter__()
    po = {}
    for m in range(2):
        for ch in range(2):
            po[(m, ch)] = psW.tile([128, 512], F32, name=f"po{m}{ch}", bufs=1)
    for h in range(HPC):
        for s in range(2):
            for m in range(2):
                for ch in range(2):
                    sl = slice(512 * ch, 512 * (ch + 1))
                    mm(po[(m, ch)][:], wout_sb[:, h, s, 128 * m:128 * (m + 1)],
                       cmb[:, h, s, sl], start=(h == 0 and s == 0),
                       stop=(h == HPC - 1 and s == 1))
    outT_sb, outT_sb_free = tc.tile([128, 2, N], F32, name="outT_sb")
    for m in range(2):
        for ch in range(2):
            sl = slice(512 * ch, 512 * (ch + 1))
            nc.scalar.copy(outT_sb[:, m, sl], po[(m, ch)][:])
            qe = nc.sync if ch == 0 else nc.gpsimd
            qe.dma_start(out=outT_d.ap()[128 * m:128 * (m + 1), sl],
                         in_=outT_sb[:, m, sl])

    psW_cm.__exit__(None, None, None)
    outT_sb_free()
    cmb_free()
    dram_cm.__exit__(None, None, None)
    att_cm.__exit__(None, None, None)
    w_free()
    x_free()
    k_od_free()
    q_od_free()
    v2_free()
    qk2_free()
    const_cm.__exit__(None, None, None)


# --------------------------------------------------------------------------
_CACHE: dict = {}


def _get_nc() -> bass.Bass:
    if "nc" not in _CACHE:
        _CACHE["nc"] = build_program()
    return _CACHE["nc"]


def _prep_core(c: int, inputs: dict) -> dict:
    b, r = c // 4, c % 4
    hs = HPC * r
    f32, f64 = np.float32, np.float64
    inp = np.asarray(inputs["inp"], f32)
    rms_w = np.asarray(inputs["rms_w"], f32)
    Wqkv = np.asarray(inputs["Wqkv"], f32)
    k_pos = np.asarray(inputs["k_pos"], f32)
    v_pos = np.asarray(inputs["v_pos"], f32)
    k_cw = np.asarray(inputs["k_cw"], f32)
    k_cb = np.asarray(inputs["k_cb"], f32)
    v_cw = np.asarray(inputs["v_cw"], f32)
    v_cb = np.asarray(inputs["v_cb"], f32)
    Ws = np.asarray(inputs["Ws"], f32)
    bs = np.asarray(inputs["bs"], f32)
    Wout = np.asarray(inputs["Wout"], f32)

    # rms_w folded into the projection weights
    cols = [Wqkv[:, p * H * DH + hs * DH: p * H * DH + (hs + HPC) * DH]
            for p in range(3)]
    w_all = np.concatenate(cols + [Ws], axis=1) * rms_w[:, None].astype(f64)
    w_all = np.ascontiguousarray(w_all).astype(NPB)

    # conv weights: per head pair, block-diagonal [64*ha + i, pair, t, 64*hb + o]
    def conv_pack(cw):
        t = np.zeros((128, 2, CB, 128), f32)
        for pair in range(2):
            for half in range(2):
                blk = cw[hs + 2 * pair + half].transpose(1, 2, 0)  # [i, t, o]
                t[64 * half:64 * half + 64, pair, :,
                  64 * half:64 * half + 64] = blk
        return np.ascontiguousarray(t).astype(NPB)

    # conv(pos) folded into the conv bias: bias_eff packed [64*hb + o, pair]
    def bias_fold(cb, cw, pos):
        cp = np.einsum("hoit,hti->ho", cw[hs:hs + HPC].astype(f64),
                       pos[hs:hs + HPC].astype(f64))
        be = cb[hs:hs + HPC] + cp                        # [4, o]
        out = np.zeros((128, 2), f32)
        for pair in range(2):
            for half in range(2):
                out[64 * half:64 * (half + 1), pair] = be[2 * pair + half]
        return np.ascontiguousarray(out)

    # output projection slabs: chunk h delivers global heads (8s+h, 8s+4+h)
    woutS = np.zeros((128, HPC, 2, 256), f32)
    for h in range(HPC):
        for s in range(2):
            ga, gb = 8 * s + h, 8 * s + 4 + h
            woutS[0:64, h, s, :] = Wout[64 * ga:64 * ga + 64,
                                        256 * r:256 * (r + 1)]
            woutS[64:128, h, s, :] = Wout[64 * gb:64 * gb + 64,
                                          256 * r:256 * (r + 1)]

    return {
        "inpT": np.ascontiguousarray(inp[b].T).astype(NPB),
        "w_all": w_all,
        "cw_k": conv_pack(k_cw),
        "cw_v": conv_pack(v_cw),
        "kcb": bias_fold(k_cb, k_cw, k_pos),
        "vcb": bias_fold(v_cb, v_cw, v_pos),
        "bs_t": np.ascontiguousarray(bs[:, None]),
        "woutS": woutS.astype(NPB),
        "ones_c": np.ones((128, 8), NPB),
        "ident_c": np.eye(128, dtype=NPB),
    }


def kernel(**inputs) -> np.ndarray:
    nc = _get_nc()
    in_maps = [_prep_core(c, inputs) for c in range(NCORES)]
    res = run_bass_kernel_spmd(nc, in_maps, list(range(NCORES)))
    out = np.zeros((B, N, DIM), np.float32)
    for c in range(NCORES):
        b, r = c // 4, c % 4
        out[b, :, 256 * r:256 * (r + 1)] = res.results[c]["outT"].T
    return out
